# revision 64
# baseline (speedup 1.0000x reference)
"""AffineLabelAttention Trainium2 kernel.

out[b, l, i, j] = W_h[l] @ head[b, i] + W_d[l] @ dep[b, j] + bias[l]

Shapes (hardcoded): head/dep [4, 1024, 768] f32, label_W [32, 1536], label_b [32].
Output [4, 32, 1024, 1024] f32 (512 MB) -> completely output-DMA-bound.

Sharding over 8 cores: core c handles batch b = c // 2 and label half
lh = c % 2 (16 labels). Each core writes a contiguous [16, 1024, 1024]
(64 MB) slice of the output.

Per-core device kernel (all fp32-exact, rel err ~5e-7 vs the reference):
  1. DMA in dep^T / head^T [768, 1024] (host pre-transposed) in 1 MB
     chunks, W halves transposed [768, 16], bias column [16, 1].
  2. PE warm-up (dummy bf16 matmuls) while inputs stream, so the fp32
     score matmuls run with HAM un-throttled (2.4 GHz, not 1.2).
  3. PE matmuls: d_score[l, j] = W_d^T.T @ dep^T (wide form; bias folded
     in during PSUM evacuation), h_score[i, l] = head^T.T @ W_h^T
     (narrow form — 8x fewer moving rows, fp32 costs 4 cycles/row).
  4. For each label l: replicate the d_score row across 128 partitions
     with a one-hot selector PE matmul (sel_l[16,128].T @ d_sb[16,N] ->
     [128,N] PSUM; sel_l[k,p] = (k==l), exact in fp32), evacuate to
     SBUF, then for each 128-row i-chunk a DVE/ACT tensor-scalar add of
     h_score[i_chunk, l] produces the out tile.
  5. 2 MB HWDGE DMAs stream the tiles to HBM. The whole kernel is
     output-DMA-bound (~400 GB/s/core sustained; ~230 us/core).

  Notes baked into the structure:
  - walrus birverifier: every compute-engine operand (SBUF or PSUM) must
    start at partition 0/32/64/96 — all per-label state is indexed along
    the free dim, never by partition offset.
  - float32r (PE fast mode) is ~tf32 precision (rel err ~1e-4) — rejected.
  - TRN2 engine instructions carry at most one semaphore wait; Bacc's
    compile() splits the rest into event-semaphores.
"""

import sys

import numpy as np

if "/opt/trn_rl_repo" not in sys.path:
    sys.path.insert(0, "/opt/trn_rl_repo")

import concourse.bass as bass
import concourse.mybir as mybir
from concourse import bacc
from concourse.bass_utils import run_bass_kernel_spmd
from concourse.tile import TileContext

B, S, D, L = 4, 1024, 768, 32
NCORES = 8
LH = L // 2          # labels per core
KCH = D // 128       # contraction chunks (6)
ICH = S // 128       # i chunks (8)
JC = S // 512        # j chunks for d matmul (2)
IC_PER = 4           # i-chunks per output tile -> 2 MB DMAs
F32 = mybir.dt.float32
BF16 = mybir.dt.bfloat16

# knobs for test harness
TRACE = False
TRACE_CORES = None
LAST_RESULTS = None

_CACHE = {}


def _build():
    # Bacc (not raw Bass): its compile() runs move_matmul_waits_to_ldweights
    # + generate_event_semaphores, required because TRN2 engine instructions
    # carry at most one semaphore wait.
    nc = bacc.Bacc("TRN2", target_bir_lowering=False, debug=False)
    headT = nc.dram_tensor("headT", [D, S], F32, kind="ExternalInput")
    depT = nc.dram_tensor("depT", [D, S], F32, kind="ExternalInput")
    whT = nc.dram_tensor("whT", [D, LH], F32, kind="ExternalInput")
    wdT = nc.dram_tensor("wdT", [D, LH], F32, kind="ExternalInput")
    # bcol: bias replicated at partition groups 0 and 32 (for the two
    # col-tiled d-score streams); sel: one-hot selectors replicated at
    # groups 0 and 32; id16: identity at partition group 64 (h stream).
    bcol = nc.dram_tensor("bcol", [48, 1], F32, kind="ExternalInput")
    sel = nc.dram_tensor("sel", [48, LH * 128], F32, kind="ExternalInput")
    id16 = nc.dram_tensor("id16", [80, LH], F32, kind="ExternalInput")
    out = nc.dram_tensor("out", [LH, S, S], F32, kind="ExternalOutput")

    # 1.5 MB input chunks: chunk c covers k-slices {3c, 3c+1, 3c+2}
    headT_v = headT[:].rearrange("(c k p) s -> c p k s", k=3, p=128)
    depT_v = depT[:].rearrange("(c k p) s -> c p k s", k=3, p=128)
    whT_v = whT[:].rearrange("(k p) l -> p k l", p=128)       # [128, 6, 16]
    wdT_v = wdT[:].rearrange("(k p) l -> p k l", p=128)
    out_v = out[:].rearrange("l (c p) j -> l p c j", p=128)   # [16, 128, 8, 1024]

    with TileContext(nc) as tc:
        with (
            tc.tile_pool(name="const", bufs=1) as cpool,
            tc.tile_pool(name="bcast", bufs=4) as bpool,
            tc.tile_pool(name="outp", bufs=7) as opool,
            tc.tile_pool(name="psum_sc", bufs=1, space="PSUM") as pss,
            tc.tile_pool(name="psum_tp", bufs=2, space="PSUM") as pst,
            tc.tile_pool(name="psum_bc", bufs=2, space="PSUM") as psb,
        ):
            depT_sb = cpool.tile([128, KCH, S], F32)
            headT_sb = cpool.tile([128, KCH, S], F32)
            whT_sb = cpool.tile([128, KCH, LH], F32)
            wdT_sb = cpool.tile([128, KCH, LH], F32)
            b_col = cpool.tile([48, 1], F32)
            sel_sb = cpool.tile([48, LH * 128], F32)  # one-hot row selectors
            id_sb = cpool.tile([80, LH], F32)         # identity @ partitions 64:80
            h_lT = cpool.tile([80, S], F32)           # h scores [l, i] @ 64:80
            h_all = cpool.tile([128, ICH, LH], F32)   # h scores, [i, l] layout
            d_sb = cpool.tile([48, S], F32)           # d+bias: jc0 @ 0:16, jc1 @ 32:48
            wu_w = cpool.tile([128, LH], BF16)        # PE warm-up operands
            wu_x = cpool.tile([128, 512], BF16)

            # Input chunks; first dep/head chunk + W first so the score
            # matmuls start as soon as chunk 0 lands.
            nc.sync.dma_start(out=depT_sb[:, 0:3, :], in_=depT_v[0])
            nc.sync.dma_start(out=wdT_sb[:], in_=wdT_v[:])
            nc.sync.dma_start(out=whT_sb[:], in_=whT_v[:])
            nc.sync.dma_start(out=headT_sb[:, 0:3, :], in_=headT_v[0])
            nc.sync.dma_start(out=depT_sb[:, 3:6, :], in_=depT_v[1])
            nc.sync.dma_start(out=headT_sb[:, 3:6, :], in_=headT_v[1])
            # needed only from the first bcast/evac (~25us in) — keep them
            # out of the way of the score-gating chunks
            nc.sync.dma_start(out=b_col[:], in_=bcol[:])
            nc.sync.dma_start(out=sel_sb[:], in_=sel[:])
            nc.sync.dma_start(out=id_sb[:], in_=id16[:])

            # All score streams are M=16, so three of them run CONCURRENTLY
            # in separate 32-column groups of the PE array (col tiling),
            # each into its own PSUM bank: d_jc0 @ partitions 0:16 (col
            # group 0), d_jc1 @ 32:48 (group 32), h_jc0 @ 64:80 (group 64).
            # h_jc1 reuses group 64 in a fourth bank afterwards (group 96
            # is PE-read-dead: operands may only start at partition
            # 0/32/64). Output partition slice must match tile_position[1].
            sc_d0 = pss.tile([128, 512], F32, name="sc_d0")
            sc_d1 = pss.tile([128, 512], F32, name="sc_d1")
            sc_h0 = pss.tile([128, 512], F32, name="sc_h0")
            sc_h1 = pss.tile([128, 512], F32, name="sc_h1")

            # PE warm-up: HAM keeps the PE clock-gated at 1.2 GHz until it
            # sees ~3.4us of sustained matmul activity. Burn cheap bf16
            # matmuls (cleared by the first real d matmul's start=True)
            # while the input DMAs stream.
            nc.vector.memset(wu_w[:], 0.0)
            nc.vector.memset(wu_x[:], 0.0)
            for _ in range(36):
                nc.tensor.matmul(sc_d0[0:LH, :], wu_w[:], wu_x[:],
                                 start=True, stop=True)

            for k in range(KCH):
                nc.tensor.matmul(
                    sc_d0[0:LH, :], wdT_sb[:, k, :],
                    depT_sb[:, k, 0:512],
                    start=(k == 0), stop=(k == KCH - 1),
                    tile_position=(0, 0),
                )
                nc.tensor.matmul(
                    sc_d1[32:32 + LH, :], wdT_sb[:, k, :],
                    depT_sb[:, k, 512:1024],
                    start=(k == 0), stop=(k == KCH - 1),
                    tile_position=(0, 32),
                )
                nc.tensor.matmul(
                    sc_h0[64:64 + LH, :], whT_sb[:, k, :],
                    headT_sb[:, k, 0:512],
                    start=(k == 0), stop=(k == KCH - 1),
                    tile_position=(0, 64),
                )
            # d evacuation (+bias) on DVE; gates the first broadcast.
            nc.vector.tensor_scalar_add(
                d_sb[0:LH, 0:512], sc_d0[0:LH, :], b_col[0:LH, :])
            nc.vector.tensor_scalar_add(
                d_sb[32:32 + LH, 512:1024], sc_d1[32:32 + LH, :],
                b_col[32:32 + LH, :])
            nc.scalar.copy(h_lT[64:64 + LH, 0:512], sc_h0[64:64 + LH, :])

            # Broadcast d row lb across 128 partitions: one-hot selector
            # matmul (exact in fp32), ACT evacuates PSUM -> SBUF. The jc0
            # stream sits at array rows 0:16, jc1 at rows 32:48 (row
            # tiling), so the two matmuls can overlap in the array.
            def bcast(lb):
                dbc = bpool.tile([128, S], F32)
                for jc in range(JC):
                    p0 = 32 * jc
                    bc_ps = psb.tile([128, 512], F32)
                    nc.tensor.matmul(
                        bc_ps[:],
                        sel_sb[p0:p0 + LH, lb * 128:(lb + 1) * 128],
                        d_sb[p0:p0 + LH, jc * 512:(jc + 1) * 512],
                        start=True,
                        stop=True,
                    )
                    nc.scalar.copy(dbc[:, jc * 512:(jc + 1) * 512], bc_ps[:])
                return dbc

            dbc_next = bcast(0)

            # h -> [i, l] layout via PE transposes of [16, 128] blocks
            # (data lives at partitions 64:80, matching identity). The
            # first output tile needs only i-chunks 0..3 (the h_jc0 half),
            # so those transposes come before the h_jc1 matmuls; h_jc1 and
            # the remaining transposes overlap the first output tiles.
            def h_transpose(ic):
                tp = pst.tile([128, LH], F32)
                nc.tensor.transpose(
                    tp[:], h_lT[64:64 + LH, ic * 128:(ic + 1) * 128],
                    id_sb[64:64 + LH, :])
                nc.scalar.copy(h_all[:, ic, :], tp[:])

            for ic in range(IC_PER):
                h_transpose(ic)

            # h_jc1 in group 64, second bank (off the first-tile path)
            for k in range(KCH):
                nc.tensor.matmul(
                    sc_h1[64:64 + LH, :], whT_sb[:, k, :],
                    headT_sb[:, k, 512:1024],
                    start=(k == 0), stop=(k == KCH - 1),
                    tile_position=(0, 64),
                )
            nc.scalar.copy(h_lT[64:64 + LH, 512:1024], sc_h1[64:64 + LH, :])
            for ic in range(IC_PER, ICH):
                h_transpose(ic)

            # Main loop: per-i-chunk adds of the h scalar onto the broadcast
            # d row; DVE takes ~5/7 of the adds, ACT the rest. bcast(lb+1)
            # is issued ahead of the adds so PE/ACT prefetch the next row.
            cnt = 0
            for lb in range(LH):
                dbc = dbc_next
                if lb + 1 < LH:
                    dbc_next = bcast(lb + 1)
                for g in range(ICH // IC_PER):
                    ot = opool.tile([128, IC_PER, S], F32)
                    for s in range(IC_PER):
                        ic = g * IC_PER + s
                        scal = h_all[:, ic, lb:lb + 1]
                        if cnt % 7 < 5:
                            nc.vector.tensor_scalar_add(ot[:, s, :], dbc[:], scal)
                        else:
                            nc.scalar.add(ot[:, s, :], dbc[:], scal)
                        cnt += 1
                    nc.sync.dma_start(
                        out=out_v[lb, :, g * IC_PER:(g + 1) * IC_PER, :],
                        in_=ot[:],
                    )
    nc.compile()
    return nc


def kernel(head, dep, label_W, label_b):
    global LAST_RESULTS
    head = np.ascontiguousarray(np.asarray(head, dtype=np.float32))
    dep = np.ascontiguousarray(np.asarray(dep, dtype=np.float32))
    label_W = np.asarray(label_W, dtype=np.float32)
    label_b = np.asarray(label_b, dtype=np.float32)

    headT = np.ascontiguousarray(head.transpose(0, 2, 1))  # [B, D, S]
    depT = np.ascontiguousarray(dep.transpose(0, 2, 1))
    whT = np.ascontiguousarray(label_W[:, :D].T)           # [D, L]
    wdT = np.ascontiguousarray(label_W[:, D:].T)           # [D, L]

    # one-hot selector sel[k, l*128 + p] = (k == l), replicated at
    # partition groups 0 and 32 (one per col-tiled d-score stream)
    sel = np.zeros((48, LH * 128), dtype=np.float32)
    for lb in range(LH):
        sel[lb, lb * 128:(lb + 1) * 128] = 1.0
    sel[32:48] = sel[0:LH]
    # identity for the h transposes, at partition group 64
    id16 = np.zeros((80, LH), dtype=np.float32)
    id16[64:80] = np.eye(LH, dtype=np.float32)

    in_maps = []
    for c in range(NCORES):
        b, lh = divmod(c, 2)
        ls = slice(lh * LH, (lh + 1) * LH)
        bc = np.zeros((48, 1), dtype=np.float32)
        bc[0:LH, 0] = label_b[ls]
        bc[32:48, 0] = label_b[ls]
        in_maps.append({
            "headT": headT[b],
            "depT": depT[b],
            "whT": np.ascontiguousarray(whT[:, ls]),
            "wdT": np.ascontiguousarray(wdT[:, ls]),
            "bcol": bc,
            "sel": sel,
            "id16": id16,
        })

    if "nc" not in _CACHE:
        _CACHE["nc"] = _build()
    nc = _CACHE["nc"]

    res = run_bass_kernel_spmd(nc, in_maps, core_ids=list(range(NCORES)),
                               trace=TRACE, trace_cores=TRACE_CORES)
    LAST_RESULTS = res

    out = np.empty((B, L, S, S), dtype=np.float32)
    for c in range(NCORES):
        b, lh = divmod(c, 2)
        out[b, lh * LH:(lh + 1) * LH] = res.results[c]["out"]
    return out


# revision 65
# speedup vs baseline: 1.0208x; 1.0208x over previous
"""AffineLabelAttention Trainium2 kernel.

out[b, l, i, j] = W_h[l] @ head[b, i] + W_d[l] @ dep[b, j] + bias[l]

Shapes (hardcoded): head/dep [4, 1024, 768] f32, label_W [32, 1536], label_b [32].
Output [4, 32, 1024, 1024] f32 (512 MB) -> completely output-DMA-bound.

Sharding over 8 cores: core c handles batch b = c // 2 and label half
lh = c % 2 (16 labels). Each core writes a contiguous [16, 1024, 1024]
(64 MB) slice of the output.

Per-core device kernel (all fp32-exact, rel err ~5e-7 vs the reference):
  1. DMA in dep^T / head^T [768, 1024] (host pre-transposed) in 1 MB
     chunks, W halves transposed [768, 16], bias column [16, 1].
  2. PE warm-up (dummy bf16 matmuls) while inputs stream, so the fp32
     score matmuls run with HAM un-throttled (2.4 GHz, not 1.2).
  3. PE matmuls: d_score[l, j] = W_d^T.T @ dep^T (wide form; bias folded
     in during PSUM evacuation), h_score[i, l] = head^T.T @ W_h^T
     (narrow form — 8x fewer moving rows, fp32 costs 4 cycles/row).
  4. For each label l: replicate the d_score row across 128 partitions
     with a one-hot selector PE matmul (sel_l[16,128].T @ d_sb[16,N] ->
     [128,N] PSUM; sel_l[k,p] = (k==l), exact in fp32), evacuate to
     SBUF, then for each 128-row i-chunk a DVE/ACT tensor-scalar add of
     h_score[i_chunk, l] produces the out tile.
  5. 2 MB HWDGE DMAs stream the tiles to HBM. The whole kernel is
     output-DMA-bound (~400 GB/s/core sustained; ~230 us/core).

  Notes baked into the structure:
  - walrus birverifier: every compute-engine operand (SBUF or PSUM) must
    start at partition 0/32/64/96 — all per-label state is indexed along
    the free dim, never by partition offset.
  - float32r (PE fast mode) is ~tf32 precision (rel err ~1e-4) — rejected.
  - TRN2 engine instructions carry at most one semaphore wait; Bacc's
    compile() splits the rest into event-semaphores.
"""

import sys

import numpy as np

if "/opt/trn_rl_repo" not in sys.path:
    sys.path.insert(0, "/opt/trn_rl_repo")

import concourse.bass as bass
import concourse.mybir as mybir
from concourse import bacc
from concourse.bass_utils import run_bass_kernel_spmd
from concourse.tile import TileContext

B, S, D, L = 4, 1024, 768, 32
NCORES = 8
LH = L // 2          # labels per core
KCH = D // 128       # contraction chunks (6)
ICH = S // 128       # i chunks (8)
JC = S // 512        # j chunks for d matmul (2)
IC_PER = 4           # i-chunks per output tile -> 2 MB DMAs
F32 = mybir.dt.float32
BF16 = mybir.dt.bfloat16

# knobs for test harness
TRACE = False
TRACE_CORES = None
LAST_RESULTS = None

_CACHE = {}


def _build():
    # Bacc (not raw Bass): its compile() runs move_matmul_waits_to_ldweights
    # + generate_event_semaphores, required because TRN2 engine instructions
    # carry at most one semaphore wait.
    nc = bacc.Bacc("TRN2", target_bir_lowering=False, debug=False)
    headT = nc.dram_tensor("headT", [D, S], F32, kind="ExternalInput")
    depT = nc.dram_tensor("depT", [D, S], F32, kind="ExternalInput")
    whT = nc.dram_tensor("whT", [D, LH], F32, kind="ExternalInput")
    wdT = nc.dram_tensor("wdT", [D, LH], F32, kind="ExternalInput")
    # bcol: bias replicated at partition groups 0 and 32 (for the two
    # col-tiled d-score streams); sel: one-hot selectors replicated at
    # groups 0 and 32; id16: identity at partition group 64 (h stream).
    bcol = nc.dram_tensor("bcol", [48, 1], F32, kind="ExternalInput")
    sel = nc.dram_tensor("sel", [48, LH * 128], F32, kind="ExternalInput")
    id16 = nc.dram_tensor("id16", [80, LH], F32, kind="ExternalInput")
    out = nc.dram_tensor("out", [LH, S, S], F32, kind="ExternalOutput")

    # 1.5 MB input chunks: chunk c covers k-slices {3c, 3c+1, 3c+2}
    headT_v = headT[:].rearrange("(c k p) s -> c p k s", k=3, p=128)
    depT_v = depT[:].rearrange("(c k p) s -> c p k s", k=3, p=128)
    whT_v = whT[:].rearrange("(k p) l -> p k l", p=128)       # [128, 6, 16]
    wdT_v = wdT[:].rearrange("(k p) l -> p k l", p=128)
    out_v = out[:].rearrange("l (c p) j -> l p c j", p=128)   # [16, 128, 8, 1024]

    with TileContext(nc) as tc:
        with (
            tc.tile_pool(name="const", bufs=1) as cpool,
            tc.tile_pool(name="bcast", bufs=4) as bpool,
            tc.tile_pool(name="outp", bufs=7) as opool,
            tc.tile_pool(name="psum_sc", bufs=1, space="PSUM") as pss,
            tc.tile_pool(name="psum_tp", bufs=2, space="PSUM") as pst,
            tc.tile_pool(name="psum_bc", bufs=2, space="PSUM") as psb,
        ):
            depT_sb = cpool.tile([128, KCH, S], F32)
            headT_sb = cpool.tile([128, KCH, S], F32)
            whT_sb = cpool.tile([128, KCH, LH], F32)
            wdT_sb = cpool.tile([128, KCH, LH], F32)
            b_col = cpool.tile([48, 1], F32)
            sel_sb = cpool.tile([48, LH * 128], F32)  # one-hot row selectors
            id_sb = cpool.tile([80, LH], F32)         # identity @ partitions 64:80
            h_lT = cpool.tile([80, S], F32)           # h scores [l, i] @ 64:80
            h_all = cpool.tile([128, ICH, LH], F32)   # h scores, [i, l] layout
            d_sb = cpool.tile([48, S], F32)           # d+bias: jc0 @ 0:16, jc1 @ 32:48
            wu_w = cpool.tile([128, LH], BF16)        # PE warm-up operands
            wu_x = cpool.tile([128, 512], BF16)

            # Input chunks; first dep/head chunk + W first so the score
            # matmuls start as soon as chunk 0 lands.
            nc.sync.dma_start(out=depT_sb[:, 0:3, :], in_=depT_v[0])
            nc.sync.dma_start(out=wdT_sb[:], in_=wdT_v[:])
            nc.sync.dma_start(out=whT_sb[:], in_=whT_v[:])
            nc.sync.dma_start(out=headT_sb[:, 0:3, :], in_=headT_v[0])
            nc.sync.dma_start(out=depT_sb[:, 3:6, :], in_=depT_v[1])
            nc.sync.dma_start(out=headT_sb[:, 3:6, :], in_=headT_v[1])
            # needed only from the first bcast/evac (~25us in) — keep them
            # out of the way of the score-gating chunks
            nc.sync.dma_start(out=b_col[:], in_=bcol[:])
            nc.sync.dma_start(out=sel_sb[:], in_=sel[:])
            nc.sync.dma_start(out=id_sb[:], in_=id16[:])

            # All score streams are M=16, so three of them run CONCURRENTLY
            # in separate 32-column groups of the PE array (col tiling),
            # each into its own PSUM bank: d_jc0 @ partitions 0:16 (col
            # group 0), d_jc1 @ 32:48 (group 32), h_jc0 @ 64:80 (group 64).
            # h_jc1 reuses group 64 in a fourth bank afterwards (group 96
            # is PE-read-dead: operands may only start at partition
            # 0/32/64). Output partition slice must match tile_position[1].
            sc_d0 = pss.tile([128, 512], F32, name="sc_d0")
            sc_d1 = pss.tile([128, 512], F32, name="sc_d1")
            sc_h0 = pss.tile([128, 512], F32, name="sc_h0")
            sc_h1 = pss.tile([128, 512], F32, name="sc_h1")

            # PE warm-up: HAM keeps the PE clock-gated at 1.2 GHz until it
            # sees ~3.4us of sustained matmul activity. Burn cheap bf16
            # matmuls (cleared by the first real d matmul's start=True)
            # while the input DMAs stream.
            nc.vector.memset(wu_w[:], 0.0)
            nc.vector.memset(wu_x[:], 0.0)
            for _ in range(36):
                nc.tensor.matmul(sc_d0[0:LH, :], wu_w[:], wu_x[:],
                                 start=True, stop=True)

            for k in range(KCH):
                nc.tensor.matmul(
                    sc_d0[0:LH, :], wdT_sb[:, k, :],
                    depT_sb[:, k, 0:512],
                    start=(k == 0), stop=(k == KCH - 1),
                    tile_position=(0, 0),
                )
                nc.tensor.matmul(
                    sc_d1[32:32 + LH, :], wdT_sb[:, k, :],
                    depT_sb[:, k, 512:1024],
                    start=(k == 0), stop=(k == KCH - 1),
                    tile_position=(0, 32),
                )
                nc.tensor.matmul(
                    sc_h0[64:64 + LH, :], whT_sb[:, k, :],
                    headT_sb[:, k, 0:512],
                    start=(k == 0), stop=(k == KCH - 1),
                    tile_position=(0, 64),
                )
            # d evacuation (+bias) on DVE; gates the first broadcast.
            nc.vector.tensor_scalar_add(
                d_sb[0:LH, 0:512], sc_d0[0:LH, :], b_col[0:LH, :])
            nc.vector.tensor_scalar_add(
                d_sb[32:32 + LH, 512:1024], sc_d1[32:32 + LH, :],
                b_col[32:32 + LH, :])
            nc.scalar.copy(h_lT[64:64 + LH, 0:512], sc_h0[64:64 + LH, :])

            # Broadcast d row lb across 128 partitions: one-hot selector
            # matmul (exact in fp32), ACT evacuates PSUM -> SBUF. The jc0
            # stream sits at array rows 0:16, jc1 at rows 32:48 (row
            # tiling), so the two matmuls can overlap in the array.
            def bcast(lb):
                dbc = bpool.tile([128, S], F32)
                for jc in range(JC):
                    p0 = 32 * jc
                    bc_ps = psb.tile([128, 512], F32)
                    nc.tensor.matmul(
                        bc_ps[:],
                        sel_sb[p0:p0 + LH, lb * 128:(lb + 1) * 128],
                        d_sb[p0:p0 + LH, jc * 512:(jc + 1) * 512],
                        start=True,
                        stop=True,
                    )
                    nc.scalar.copy(dbc[:, jc * 512:(jc + 1) * 512], bc_ps[:])
                return dbc

            dbc_next = bcast(0)

            # h -> [i, l] layout via PE transposes of [16, 128] blocks
            # (data lives at partitions 64:80, matching identity). The
            # first output tile needs only i-chunks 0..3 (the h_jc0 half),
            # so those transposes come before the h_jc1 matmuls; h_jc1 and
            # the remaining transposes overlap the first output tiles.
            def h_transpose(ic):
                tp = pst.tile([128, LH], F32)
                nc.tensor.transpose(
                    tp[:], h_lT[64:64 + LH, ic * 128:(ic + 1) * 128],
                    id_sb[64:64 + LH, :])
                nc.scalar.copy(h_all[:, ic, :], tp[:])

            for ic in range(IC_PER):
                h_transpose(ic)

            # h_jc1 in group 64, second bank (off the first-tile path)
            for k in range(KCH):
                nc.tensor.matmul(
                    sc_h1[64:64 + LH, :], whT_sb[:, k, :],
                    headT_sb[:, k, 512:1024],
                    start=(k == 0), stop=(k == KCH - 1),
                    tile_position=(0, 64),
                )
            nc.scalar.copy(h_lT[64:64 + LH, 512:1024], sc_h1[64:64 + LH, :])
            for ic in range(IC_PER, ICH):
                h_transpose(ic)

            # Main loop: per-i-chunk adds of the h scalar onto the broadcast
            # d row; DVE takes ~5/7 of the adds, ACT the rest. bcast(lb+1)
            # is issued ahead of the adds so PE/ACT prefetch the next row.
            cnt = 0
            for lb in range(LH):
                dbc = dbc_next
                if lb + 1 < LH:
                    dbc_next = bcast(lb + 1)
                # smaller first tile on l=0 so the first DMA launches as
                # early as possible (2 adds instead of 4 gate it)
                groups = [(0, 2), (2, 2), (4, 4)] if lb == 0 else \
                         [(0, 4), (4, 4)]
                for g0, gn in groups:
                    ot = opool.tile([128, IC_PER, S], F32)
                    for s in range(gn):
                        ic = g0 + s
                        scal = h_all[:, ic, lb:lb + 1]
                        if cnt % 7 < 5:
                            nc.vector.tensor_scalar_add(ot[:, s, :], dbc[:], scal)
                        else:
                            nc.scalar.add(ot[:, s, :], dbc[:], scal)
                        cnt += 1
                    nc.sync.dma_start(
                        out=out_v[lb, :, g0:g0 + gn, :],
                        in_=ot[:, 0:gn, :],
                    )
    nc.compile()
    return nc


def kernel(head, dep, label_W, label_b):
    global LAST_RESULTS
    head = np.ascontiguousarray(np.asarray(head, dtype=np.float32))
    dep = np.ascontiguousarray(np.asarray(dep, dtype=np.float32))
    label_W = np.asarray(label_W, dtype=np.float32)
    label_b = np.asarray(label_b, dtype=np.float32)

    headT = np.ascontiguousarray(head.transpose(0, 2, 1))  # [B, D, S]
    depT = np.ascontiguousarray(dep.transpose(0, 2, 1))
    whT = np.ascontiguousarray(label_W[:, :D].T)           # [D, L]
    wdT = np.ascontiguousarray(label_W[:, D:].T)           # [D, L]

    # one-hot selector sel[k, l*128 + p] = (k == l), replicated at
    # partition groups 0 and 32 (one per col-tiled d-score stream)
    sel = np.zeros((48, LH * 128), dtype=np.float32)
    for lb in range(LH):
        sel[lb, lb * 128:(lb + 1) * 128] = 1.0
    sel[32:48] = sel[0:LH]
    # identity for the h transposes, at partition group 64
    id16 = np.zeros((80, LH), dtype=np.float32)
    id16[64:80] = np.eye(LH, dtype=np.float32)

    in_maps = []
    for c in range(NCORES):
        b, lh = divmod(c, 2)
        ls = slice(lh * LH, (lh + 1) * LH)
        bc = np.zeros((48, 1), dtype=np.float32)
        bc[0:LH, 0] = label_b[ls]
        bc[32:48, 0] = label_b[ls]
        in_maps.append({
            "headT": headT[b],
            "depT": depT[b],
            "whT": np.ascontiguousarray(whT[:, ls]),
            "wdT": np.ascontiguousarray(wdT[:, ls]),
            "bcol": bc,
            "sel": sel,
            "id16": id16,
        })

    if "nc" not in _CACHE:
        _CACHE["nc"] = _build()
    nc = _CACHE["nc"]

    res = run_bass_kernel_spmd(nc, in_maps, core_ids=list(range(NCORES)),
                               trace=TRACE, trace_cores=TRACE_CORES)
    LAST_RESULTS = res

    out = np.empty((B, L, S, S), dtype=np.float32)
    for c in range(NCORES):
        b, lh = divmod(c, 2)
        out[b, lh * LH:(lh + 1) * LH] = res.results[c]["out"]
    return out
